# revision 2
# baseline (speedup 1.0000x reference)
"""BiLSTM + additive attention pooling on 8 TRN2 cores — time-sharded v2.

Design (vs the batch-sharded baseline):
- Each core owns ONE time chunk of 256 steps and runs BOTH directions
  (fwd + bwd chains) over the FULL batch of 64 rows, interleaved step by
  step so ACT/DVE/PE work of the two chains overlaps.
- The LSTM state has a short memory horizon (forget gate ~sigmoid(z),
  E[log f] ~ -0.7), so each chain warms up for W=32 steps from h=c=0
  before its real window: carry-in error ~exp(-0.7*32) ~ 1e-10.
- N=64 batch columns per instruction amortizes the large fixed
  per-instruction costs (ACT ~250ns, DVE ~150ns) 4x better than the
  16-row baseline.
- Attention (softmax over time of att_w . tanh(h_f+h_b), then weighted
  sum) is computed per core over its own chunk only; the tiny partial
  numerator/denominator [129,128] f32 are summed on the HOST (no
  collectives at all).

Gate trick: gate order host-permuted to (i,f,o,g) and g columns
pre-scaled by 2 so tanh(g) = 2*sigmoid(2g)-1 comes out of the same
sigmoid pass as i,f,o: c' = f*c + 2*i*sg - i;  h = o*tanh(c').

Env toggles: V2_REC = bf16|f8  (f8 = DoubleRow fp8 recurrent matmuls)
             V2_CDT = f32|bf16 (cell-update dtype)
"""
import os
import sys

sys.path.insert(0, "/opt/trn_rl_repo")

REC = os.environ.get("V2_REC", "bf16")   # 'bf16' | 'f8'
CDT = os.environ.get("V2_CDT", "f32")    # 'f32' | 'bf16'

import numpy as np
import ml_dtypes

from concourse import bass, bacc, tile, mybir
from concourse.bass_utils import run_bass_kernel_spmd

F32 = mybir.dt.float32
BF16 = mybir.dt.bfloat16
F8 = mybir.dt.float8e4
BF16_NP = ml_dtypes.bfloat16
F8_NP = ml_dtypes.float8_e4m3

B, T, D, H = 64, 2048, 128, 256
G4 = 4 * H              # 1024
NCORES = 8
CH = T // NCORES        # 256 own steps per core
W = 32                  # warmup steps
TS = CH + W             # 288 steps per chain
S = 16                  # steps per half-chunk
NIT = TS // (2 * S)     # 9 loop iterations
XLEN = TS + S           # x padded by one half-chunk for the proj lookahead
TCH = 32                # attention time chunk
NTC = CH // TCH
SR, SH = 16.0, 4.0      # f8 scales for r and h
SCALE = SR * SH if REC == "f8" else 1.0

Sigmoid = mybir.ActivationFunctionType.Sigmoid
Tanh = mybir.ActivationFunctionType.Tanh
Exp = mybir.ActivationFunctionType.Exp
MUL = mybir.AluOpType.mult
DR = mybir.MatmulPerfMode.DoubleRow

CD = F32 if CDT == "f32" else BF16

_CACHE = {}


def _build():
    nc = bacc.Bacc("TRN2", target_bir_lowering=False, debug=False,
                   num_devices=NCORES)

    xf_ext = nc.dram_tensor("xf", [D, XLEN, B], BF16, kind="ExternalInput")
    xb_ext = nc.dram_tensor("xb", [D, XLEN, B], BF16, kind="ExternalInput")
    k_ext = nc.dram_tensor("k", [2, D, G4], BF16, kind="ExternalInput")
    rdt = F8 if REC == "f8" else BF16
    r_ext = nc.dram_tensor("r", [2, 128, 2, G4], rdt, kind="ExternalInput")
    aw_ext = nc.dram_tensor("attw", [128, 2, 1], BF16, kind="ExternalInput")
    out_ext = nc.dram_tensor("out", [129, 128], F32, kind="ExternalOutput")

    hist_kind = ("ExternalOutput" if os.environ.get("V2_DBG") else "Internal")
    histf = nc.dram_tensor("histf", [128, 2, TS, B], BF16, kind=hist_kind)
    histb = nc.dram_tensor("histb", [128, 2, TS, B], BF16, kind=hist_kind)
    if os.environ.get("V2_DBG"):
        gdump = nc.dram_tensor("gdump", [128, 2, 4, B], F32,
                               kind="ExternalOutput")
    e_dram = nc.dram_tensor("e_dram", [CH, B], BF16, kind=hist_kind)
    sc_dram = nc.dram_tensor("sc_dram", [1, CH, B], F32)
    if os.environ.get("V2_DBG"):
        scdump = nc.dram_tensor("scdump", [B, CH], F32, kind="ExternalOutput")
        scdump2 = nc.dram_tensor("scdump2", [1, CH, B], F32,
                                 kind="ExternalOutput")
        dendump = nc.dram_tensor("dendump", [B, 1], F32,
                                 kind="ExternalOutput")

    with tile.TileContext(nc) as tc, \
         tc.tile_pool(name="const", bufs=1) as constp:
        k_sb = constp.tile([D, 2, G4], BF16)
        r_sb = constp.tile([128, 2, 2, G4], rdt)
        aw_sb = constp.tile([128, 2, 1], BF16)
        nc.sync.dma_start(k_sb[:], k_ext.ap().rearrange("x d m -> d x m"))
        nc.sync.dma_start(r_sb[:], r_ext.ap().rearrange("x p kj m -> p x kj m"))
        nc.sync.dma_start(aw_sb[:], aw_ext[:])

        with (
            tc.tile_pool(name="state", bufs=1) as st,
            tc.tile_pool(name="zp", bufs=1, space="PSUM") as zp,
        ):
            # per chain X (0=fwd, 1=bwd)
            cst = [st.tile([128, 2, B], CD, tag=f"cst{X}", name=f"cst{X}") for X in range(2)]
            gates = [st.tile([128, 2, 4, B], CD, tag=f"g{X}", name=f"g{X}") for X in range(2)]
            th = [st.tile([128, 2, B], CD, tag=f"th{X}", name=f"th{X}") for X in range(2)]
            tmp = [st.tile([128, 2, B], CD, tag=f"tm{X}", name=f"tm{X}") for X in range(2)]
            hh = [[st.tile([128, 2, S, B], BF16, tag=f"hh{X}{h}", name=f"hh{X}{h}")
                   for h in range(2)] for X in range(2)]
            xt = [[st.tile([D, S, B], BF16, tag=f"xt{X}{h}", name=f"xt{X}{h}")
                   for h in range(2)] for X in range(2)]
            z = [[zp.tile([128, 2, 4, B], F32, tag=f"z{X}{p}", name=f"z{X}{p}")
                  for p in range(2)] for X in range(2)]
            if REC == "f8":
                h8 = [st.tile([128, 2, 2, B], F8, tag=f"h8{X}", name=f"h8{X}")
                      for X in range(2)]

            for X in range(2):
                nc.vector.memset(cst[X][:], 0.0)
                nc.vector.memset(hh[X][1][:, :, S - 1, :], 0.0)
                if REC == "f8":
                    nc.vector.memset(h8[X][:, 1, :, :], 0.0)

            def emit_proj(X, half, s):
                # standalone accumulation groups (start+stop per region);
                # the rec matmuls later accumulate on top (start=False)
                zt = z[X][s % 2]
                for j in range(2):
                    for g in range(4):
                        m0 = g * 256 + j * 128
                        nc.tensor.matmul(zt[:, j, g, :],
                                         k_sb[:, X, m0:m0 + 128],
                                         xt[X][half][:, s, :],
                                         start=True, stop=True,
                                         skip_group_check=True)

            def emit_rec(X, half, s):
                # region-contiguous groups: kj inner so a bank never has
                # two interleaved open groups (that corrupts PSUM)
                zt = z[X][s % 2]
                if REC == "f8":
                    hp = h8[X][:, (s - 1) % 2, :, :]
                    for j in range(2):
                        for g in range(4):
                            m0 = g * 256 + j * 128
                            nc.tensor.matmul(zt[:, j, g, :],
                                             r_sb[:, X, :, m0:m0 + 128],
                                             hp, start=False, stop=True,
                                             perf_mode=DR,
                                             skip_group_check=True)
                else:
                    if s > 0:
                        hp = hh[X][half][:, :, s - 1, :]
                    else:
                        hp = hh[X][1 - half][:, :, S - 1, :]
                    for j in range(2):
                        for g in range(4):
                            m0 = g * 256 + j * 128
                            for kj in range(2):
                                nc.tensor.matmul(
                                    zt[:, j, g, :],
                                    r_sb[:, X, kj, m0:m0 + 128],
                                    hp[:, kj, :], start=False,
                                    stop=(kj == 1),
                                    skip_group_check=True)

            slot_no = [0]

            def emit_projrec_seq(X, half, s):
                # non-interleaved groups: per region proj, rec, rec(stop)
                zt = z[X][s % 2]
                if s > 0:
                    hp = hh[X][half][:, :, s - 1, :]
                else:
                    hp = hh[X][1 - half][:, :, S - 1, :]
                for j in range(2):
                    for g in range(4):
                        m0 = g * 256 + j * 128
                        nc.tensor.matmul(zt[:, j, g, :],
                                         k_sb[:, X, m0:m0 + 128],
                                         xt[X][half][:, s, :],
                                         start=True, stop=False,
                                         skip_group_check=True)
                        for kj in range(2):
                            nc.tensor.matmul(
                                zt[:, j, g, :],
                                r_sb[:, X, kj, m0:m0 + 128],
                                hp[:, kj, :], start=False,
                                stop=(kj == 1),
                                skip_group_check=True)

            SEQGRP = os.environ.get("V2_MODE", "seq") == "seq"

            def emit_slot(half, s, nxt):
                # nxt = (half', s') of the step whose proj/rec we issue,
                # or None at the very end
                if nxt is not None and not SEQGRP:
                    for X in range(2):
                        emit_proj(X, nxt[0], nxt[1])
                for X in range(2):
                    nc.scalar.activation(gates[X][:], z[X][s % 2][:],
                                         Sigmoid, scale=1.0 / SCALE)
                if os.environ.get("V2_DBG") and os.environ.get("V2_UNROLL") \
                        and slot_no[0] == int(os.environ.get("V2_DBG_SLOT", "0")):
                    nc.sync.dma_start(gdump.ap()[:], gates[0][:])
                slot_no[0] += 1
                for X in range(2):
                    # c' = f*c + 2*i*sg - i
                    nc.vector.scalar_tensor_tensor(
                        tmp[X][:], gates[X][:, :, 3, :], 2.0,
                        gates[X][:, :, 0, :], MUL, MUL)
                    nc.vector.tensor_mul(cst[X][:], gates[X][:, :, 1, :],
                                         cst[X][:])
                    nc.vector.tensor_add(cst[X][:], cst[X][:], tmp[X][:])
                    nc.vector.tensor_sub(cst[X][:], cst[X][:],
                                         gates[X][:, :, 0, :])
                for X in range(2):
                    nc.scalar.activation(th[X][:], cst[X][:], Tanh)
                for X in range(2):
                    if REC == "f8":
                        nc.vector.scalar_tensor_tensor(
                            h8[X][:, s % 2, :, :], th[X][:], SH,
                            gates[X][:, :, 2, :], MUL, MUL)
                        nc.gpsimd.scalar_tensor_tensor(
                            hh[X][half][:, :, s, :], th[X][:], 1.0,
                            gates[X][:, :, 2, :], MUL, MUL)
                    else:
                        nc.vector.tensor_mul(hh[X][half][:, :, s, :],
                                             gates[X][:, :, 2, :], th[X][:])
                if nxt is not None:
                    for X in range(2):
                        if SEQGRP:
                            emit_projrec_seq(X, nxt[0], nxt[1])
                        else:
                            emit_rec(X, nxt[0], nxt[1])

            # prologue: x for half 0, proj+rec of step 0
            for X in range(2):
                nc.sync.dma_start(xt[X][0][:], (xf_ext if X == 0 else
                                                xb_ext)[:, 0:S, :])
            for X in range(2):
                if SEQGRP:
                    emit_projrec_seq(X, 0, 0)
                else:
                    emit_proj(X, 0, 0)
            if not SEQGRP:
                for X in range(2):
                    emit_rec(X, 0, 0)

            hist = [histf, histb]
            xe = [xf_ext, xb_ext]

            def loop_body(i):
                t0 = i * (2 * S)
                for X in range(2):
                    nc.sync.dma_start(xt[X][1][:],
                                      xe[X][:, bass.ds(t0 + S, S), :])
                for s in range(S):
                    emit_slot(0, s, (0, s + 1) if s < S - 1 else (1, 0))
                for X in range(2):
                    nc.sync.dma_start(hist[X][:, :, bass.ds(t0, S), :],
                                      hh[X][0][:])
                for X in range(2):
                    nc.sync.dma_start(xt[X][0][:],
                                      xe[X][:, bass.ds(t0 + 2 * S, S), :])
                for s in range(S):
                    emit_slot(1, s, (1, s + 1) if s < S - 1 else (0, 0))
                for X in range(2):
                    nc.sync.dma_start(hist[X][:, :, bass.ds(t0 + S, S), :],
                                      hh[X][1][:])

            if os.environ.get("V2_UNROLL"):
                for i in range(NIT):
                    loop_body(i)
            else:
                with tc.For_i(0, NIT, 1,
                              hint_engines=(mybir.EngineType.PE,
                                            mybir.EngineType.Activation,
                                            mybir.EngineType.DVE)) as i:
                    loop_body(i)

        # ---- attention over this core's own chunk (both dirs local)
        with (
            tc.tile_pool(name="att1", bufs=2) as att1,
            tc.tile_pool(name="scpp", bufs=2, space="PSUM") as scpp,
            tc.tile_pool(name="smx", bufs=1) as smx,
        ):
            hs_all = smx.tile([128, 2, CH, B], BF16)
            sc_sb = smx.tile([1, CH, B], F32)
            for c in range(NTC):
                t0 = c * TCH
                hf = att1.tile([128, 2, TCH, B], BF16, tag="hf")
                hb = att1.tile([128, 2, TCH, B], BF16, tag="hb")
                m = att1.tile([128, 2, TCH, B], BF16, tag="m")
                nc.sync.dma_start(hf[:], histf[:, :, W + t0:W + t0 + TCH, :])
                rstart = (TS - 1) - t0
                rstop = rstart - TCH
                rsl = slice(rstart, None if rstop < 0 else rstop, -1)
                nc.sync.dma_start(hb[:], histb[:, :, rsl, :])
                nc.vector.tensor_add(hs_all[:, :, t0:t0 + TCH, :],
                                     hf[:], hb[:])
                nc.scalar.activation(m[:], hs_all[:, :, t0:t0 + TCH, :],
                                     Tanh)
                nsub = TCH // 8
                for sub in range(nsub):
                    scp = scpp.tile([1, 8, B], F32, tag="scp")
                    for j in range(2):
                        nc.tensor.matmul(
                            scp[:], aw_sb[:, j, :],
                            m[:, j, sub * 8:(sub + 1) * 8, :],
                            start=(j == 0), stop=(j == 1))
                    nc.vector.tensor_copy(
                        sc_sb[:, t0 + sub * 8:t0 + (sub + 1) * 8, :],
                        scp[:])

            # softmax over the local chunk (no max-sub: scores are small)
            scT = smx.tile([B, CH], F32)
            eT = smx.tile([B, CH], BF16)
            den = smx.tile([B, 1], F32)
            nc.sync.dma_start(sc_dram.ap()[:], sc_sb[:])
            nc.sync.dma_start(scT[:],
                              sc_dram.ap()[0].rearrange("t b -> b t"))
            if os.environ.get("V2_DBG"):
                nc.sync.dma_start(scdump.ap()[:], scT[:])
                nc.sync.dma_start(scdump2.ap()[:], sc_sb[:])
            nc.scalar.activation(eT[:], scT[:], Exp, accum_out=den[:])
            if os.environ.get("V2_DBG"):
                nc.sync.dma_start(dendump.ap()[:], den[:])
            nc.sync.dma_start(e_dram.ap().rearrange("t b -> b t"), eT[:])

            rtot = smx.tile([128, 2, B], F32)
            nc.vector.memset(rtot[:], 0.0)
            for c in range(NTC):
                t0 = c * TCH
                abc = att1.tile([128, TCH, B], BF16, tag="abc")
                nc.sync.dma_start(
                    abc[:], e_dram.ap()[t0:t0 + TCH, :]
                    .partition_broadcast(128))
                for j in range(2):
                    wm = att1.tile([128, TCH, B], BF16, tag="wm")
                    racc = att1.tile([128, B], F32, tag="racc")
                    nc.vector.tensor_mul(wm[:], hs_all[:, j, t0:t0 + TCH, :],
                                         abc[:])
                    nc.vector.tensor_reduce(
                        racc[:], wm[:].rearrange("p t b -> p b t"),
                        mybir.AxisListType.X, mybir.AluOpType.add)
                    nc.vector.tensor_add(rtot[:, j, :], rtot[:, j, :],
                                         racc[:])
            nc.sync.dma_start(
                out_ext.ap()[0:128].rearrange("p (j b) -> p j b", j=2),
                rtot[:])
            nc.sync.dma_start(
                out_ext.ap()[128:129, 0:B].rearrange("o b -> b o"), den[:])

    nc.compile()
    return nc


GATE_PERM = np.concatenate([np.arange(0, 512), np.arange(768, 1024),
                            np.arange(512, 768)])  # (i,f,g,o) -> (i,f,o,g)


def _prep_weights(k, r, b):
    assert np.abs(b).max() < 1e-6, "bias assumed zero"
    k = k[:, GATE_PERM].copy()
    r = r[:, GATE_PERM].copy()
    k[:, 768:1024] *= 2.0   # tanh(g) = 2*sigmoid(2g) - 1
    r[:, 768:1024] *= 2.0
    k *= SCALE
    # r: [256, 1024] -> [128 p, 2 kj, 1024]
    r = r.reshape(2, 128, G4).transpose(1, 0, 2)
    if REC == "f8":
        r = (r * SR).astype(F8_NP)
    else:
        r = r.astype(BF16_NP)
    return np.ascontiguousarray(k.astype(BF16_NP)), np.ascontiguousarray(r)


def run(inputs, trace=False):
    if "nc" not in _CACHE:
        _CACHE["nc"] = _build()
    nc = _CACHE["nc"]
    x = np.asarray(inputs["x"], np.float32)

    kf, rf = _prep_weights(np.asarray(inputs["k_fwd"], np.float32),
                           np.asarray(inputs["r_fwd"], np.float32),
                           np.asarray(inputs["b_fwd"], np.float32))
    kb, rb = _prep_weights(np.asarray(inputs["k_bwd"], np.float32),
                           np.asarray(inputs["r_bwd"], np.float32),
                           np.asarray(inputs["b_bwd"], np.float32))
    k_both = np.ascontiguousarray(np.stack([kf, kb]))
    r_both = np.ascontiguousarray(np.stack([rf, rb]))
    aw = np.ascontiguousarray(
        np.asarray(inputs["att_w"], np.float32).reshape(2, 128).T.reshape(
            128, 2, 1).astype(BF16_NP))

    # padded x^T: [D, W + T + W + S, B]
    xP = np.zeros((D, W + T + W + S, B), BF16_NP)
    xP[:, W:W + T, :] = x.transpose(2, 1, 0).astype(BF16_NP)

    in_maps = []
    for c in range(NCORES):
        xf = np.ascontiguousarray(xP[:, 256 * c:256 * c + XLEN, :])
        xb = np.ascontiguousarray(
            xP[:, 256 * c + S:256 * c + S + XLEN, :][:, ::-1, :])
        in_maps.append({"xf": xf, "xb": xb, "k": k_both, "r": r_both,
                        "attw": aw})

    res = run_bass_kernel_spmd(nc, in_maps, list(range(NCORES)), trace=trace)
    parts = [res.results[c]["out"] for c in range(NCORES)]
    tot = np.sum(np.stack(parts), axis=0)
    num = tot[0:128].reshape(128, 2, B)
    den = tot[128, 0:B]
    out = np.tanh(num / den[None, None, :]).transpose(2, 1, 0).reshape(B, H)
    return np.ascontiguousarray(out.astype(np.float32)), res


def kernel(**inputs):
    out, _ = run(inputs)
    return out
